# revision 4
# baseline (speedup 1.0000x reference)
"""Trainium2 Bass kernel for nn_MixLinear_GEMM (int4-dequant -> dynamic fp8 GEMM + outlier correction).

Self-contained: kernel(**inputs) takes full inputs, shards across 8 NeuronCores
(tensor-parallel along out_features N), runs one SPMD Bass kernel with
collectives (AllReduce for global maxes, chunked AllGather for fp8-quantized
x^T), and returns the full [M, N] float32 output.

Math notes:
 - reference quantizes to OCP float8_e4m3fn (max 448). TRN2's fp8e4 has max 240,
   so we quantize v/2 instead (max 224) and fold the 2x2 into the output scale.
   For this problem's data every |v| >= 448*min_scale/max|W| >> 2^-5, so the
   halved TRN rounding is bit-identical to e4m3fn rounding of v.
 - y = (Xq@Wq^T)*(sx*sw) + bias + x[:,ind]@wc^T  is computed as
   psum = sum_k (Xq/2)(Wq/2) + (xg_aug @ wct_aug^T) / s4,   s4 = 4*sx*sw
   y = psum * s4         (bias is an extra column of the correction GEMM)
"""
import sys

if "/opt/trn_rl_repo" not in sys.path:
    sys.path.insert(0, "/opt/trn_rl_repo")

import numpy as np

import concourse.bass as bass
import concourse.mybir as mybir
import concourse.tile as tile
from concourse import bacc, bass_isa
from concourse.bass_utils import run_bass_kernel_spmd
from concourse.masks import make_identity

F32 = mybir.dt.float32
I32 = mybir.dt.int32
U8 = mybir.dt.uint8
FP8 = mybir.dt.float8e4
ALU = mybir.AluOpType

CORES = 8
GROUP = 128
FP8_MAX_REF = 448.0  # reference e4m3fn max
FP8_HALF_MAX = 224.0  # what we scale the max element to (TRN fp8e4 max is 240)


def build_kernel(M=4096, K=8192, N=8192, CAUG=384):
    """Build the SPMD Bass graph (one graph, runs on all 8 cores)."""
    NL = N // CORES          # local out_features
    MSL = M // CORES         # local x row-slice
    KP = K // 128            # number of 128-wide k' chunks
    NWIN = KP // 8           # qwt row windows of 128 (each yields 8 chunks)
    MT = MSL // 128          # local m-subtiles
    NB = max(1, NL // 512)   # psum banks per m-tile
    NBW = min(NL, 512)       # psum bank width
    K4 = min(K, 4096)        # x staging tile width
    WPK4 = K4 // 1024        # windows per x staging tile
    NXT = K // K4            # x staging tiles per m-tile
    assert KP % 8 == 0 and MSL % 128 == 0 and NL % NBW == 0 and CAUG % 128 == 0

    nc = bacc.Bacc("TRN2", target_bir_lowering=False, debug=False, num_devices=CORES)

    xs = nc.declare_dram_parameter("xs", [MSL, K], F32, isOutput=False)
    qwt = nc.declare_dram_parameter("qwt", [K // 8, NL], I32, isOutput=False)
    sct = nc.declare_dram_parameter("sct", [K // GROUP, NL], F32, isOutput=False)
    xgt = nc.declare_dram_parameter("xgt", [CAUG, M], F32, isOutput=False)
    wct = nc.declare_dram_parameter("wct", [CAUG, NL], F32, isOutput=False)
    y = nc.declare_dram_parameter("y", [M, NL], F32, isOutput=True)

    with tile.TileContext(nc) as tc:
        with (
            tc.tile_pool(name="const", bufs=1) as constp,
            tc.tile_pool(name="wt", bufs=1) as wtp,
            tc.tile_pool(name="stream", bufs=2) as streamp,
            tc.tile_pool(name="xa", bufs=2) as xap,
            tc.tile_pool(name="xt", bufs=3) as xtp,
            tc.tile_pool(name="ysb", bufs=2) as ysbp,
            tc.tile_pool(name="xgc", bufs=4) as xgcp,
            tc.tile_pool(name="psum_t", bufs=4, space="PSUM") as psumt,
            tc.tile_pool(name="psum_mm", bufs=4, space="PSUM") as psummm,
            tc.tile_pool(name="dram", bufs=1, space="DRAM") as dram,
        ):
            ident = constp.tile([128, 128], F32, tag="ident")
            make_identity(nc, ident[:])

            # persistent accumulators / scalars
            xmax_cols = constp.tile([128, MT * NXT], F32, tag="xmax")
            wmax_cols = constp.tile([128, KP], F32, tag="wmax")
            gmax_sb = constp.tile([128, 2], F32, tag="gmax")
            rx = constp.tile([128, 1], F32, tag="rx")
            rw = constp.tile([128, 1], F32, tag="rw")
            s4 = constp.tile([128, 1], F32, tag="s4")
            invs4 = constp.tile([128, 1], F32, tag="invs4")
            tmp1 = constp.tile([128, 1], F32, tag="tmp1")
            lmax2 = constp.tile([128, 2], F32, tag="lmax2")
            lred = constp.tile([128, 2], F32, tag="lred")

            # correction weights (scaled by 1/s4 once it is known)
            wct_sb = []
            for q in range(CAUG // 128):
                t = constp.tile([128, NL], F32, tag=f"wct{q}")
                nc.sync.dma_start(out=t[:], in_=wct[q * 128:(q + 1) * 128, :])
                wct_sb.append(t)

            # ---- phase A-w: local max |W| ----
            def load_window_planes(w):
                """DMA window w of qwt, unpack to lo/hi nibble planes (packed int32)."""
                qa = streamp.tile([128, NL], I32, tag="qa")
                nc.sync.dma_start(out=qa[:], in_=qwt[w * 128:(w + 1) * 128, :])
                lo = streamp.tile([128, NL], I32, tag="lo")
                hi = streamp.tile([128, NL], I32, tag="hi")
                nc.vector.tensor_scalar(lo[:], qa[:], 0x0F0F0F0F, None, ALU.bitwise_and)
                nc.vector.tensor_scalar(hi[:], qa[:], 4, None, ALU.logical_shift_right)
                nc.vector.tensor_scalar(hi[:], hi[:], 0x0F0F0F0F, None, ALU.bitwise_and)
                return lo, hi

            def load_srep(w):
                """Scale rows for window w, replicated 16x across partitions."""
                srep = streamp.tile([128, NL], F32, tag="srep")
                for g in range(8):
                    nc.sync.dma_start(
                        out=srep[g * 16:(g + 1) * 16, :],
                        in_=sct[w * 8 + g:w * 8 + g + 1, :].broadcast_to([16, NL]),
                    )
                return srep

            def plane(lo, hi, j):
                src = lo if j % 2 == 0 else hi
                b = j // 2
                return src[:].bitcast(U8)[:, b::4]

            for w in range(NWIN):
                lo, hi = load_window_planes(w)
                srep = load_srep(w)
                for j in range(8):
                    wtmp = streamp.tile([128, NL], F32, tag="wtmp")
                    nc.vector.scalar_tensor_tensor(
                        out=wtmp[:], in0=plane(lo, hi, j), scalar=-8.0, in1=srep[:],
                        op0=ALU.add, op1=ALU.mult,
                    )
                    nc.vector.tensor_reduce(
                        out=wmax_cols[:, w * 8 + j: w * 8 + j + 1], in_=wtmp[:],
                        axis=mybir.AxisListType.X, op=ALU.max, apply_absolute_value=True,
                    )

            # ---- phase A-x: local max |x| ----
            for mt in range(MT):
                for h in range(NXT):
                    xa = xap.tile([128, K4], F32, tag="xa")
                    nc.sync.dma_start(
                        out=xa[:], in_=xs[mt * 128:(mt + 1) * 128, h * K4:(h + 1) * K4]
                    )
                    col = mt * NXT + h
                    nc.vector.tensor_reduce(
                        out=xmax_cols[:, col:col + 1], in_=xa[:],
                        axis=mybir.AxisListType.X, op=ALU.max, apply_absolute_value=True,
                    )

            # ---- combine local maxes, AllReduce(max) ----
            nc.vector.tensor_reduce(
                out=lmax2[:, 0:1], in_=xmax_cols[:], axis=mybir.AxisListType.X,
                op=ALU.max, apply_absolute_value=True,
            )
            nc.vector.tensor_reduce(
                out=lmax2[:, 1:2], in_=wmax_cols[:], axis=mybir.AxisListType.X,
                op=ALU.max, apply_absolute_value=True,
            )
            nc.gpsimd.partition_all_reduce(lred[:], lmax2[:], 128, bass_isa.ReduceOp.max)
            ar_in = dram.tile([1, 2], F32, tag="ar_in")
            ar_out = dram.tile([1, 2], F32, tag="ar_out")
            nc.sync.dma_start(out=ar_in[:], in_=lred[0:1, :])
            nc.gpsimd.collective_compute(
                "AllReduce", ALU.max,
                replica_groups=[list(range(CORES))],
                ins=[ar_in[:].opt()], outs=[ar_out[:].opt()],
            )
            g1 = constp.tile([1, 2], F32, tag="g1")
            nc.sync.dma_start(out=g1[:], in_=ar_out[:])
            nc.gpsimd.partition_broadcast(gmax_sb[:], g1[0:1, :], channels=128)

            # derived scalars:  rx = 224/gx,  rw = 224/gw,
            # s4 = 4*sx*sw = gx*gw/(448*448/4) = gx*gw/50176
            nc.vector.reciprocal(tmp1[:], gmax_sb[:, 0:1])
            nc.vector.tensor_scalar(rx[:], tmp1[:], FP8_HALF_MAX, None, ALU.mult)
            nc.vector.reciprocal(tmp1[:], gmax_sb[:, 1:2])
            nc.vector.tensor_scalar(rw[:], tmp1[:], FP8_HALF_MAX, None, ALU.mult)
            nc.vector.tensor_tensor(s4[:], gmax_sb[:, 0:1], gmax_sb[:, 1:2], ALU.mult)
            nc.vector.tensor_scalar(s4[:], s4[:], 1.0 / 50176.0, None, ALU.mult)
            nc.vector.reciprocal(invs4[:], s4[:])

            # scale correction weights by 1/s4
            for q in range(CAUG // 128):
                nc.vector.tensor_scalar(wct_sb[q][:], wct_sb[q][:], invs4[:], None, ALU.mult)

            # ---- phase B-W: quantize weights into resident Wt tiles ----
            wt_sb = []
            for w in range(NWIN):
                wt_w = wtp.tile([128, 8, NL], FP8, tag=f"wt{w}")
                wt_sb.append(wt_w)
                lo, hi = load_window_planes(w)
                srep = load_srep(w)
                nc.vector.tensor_scalar(srep[:], srep[:], rw[:], None, ALU.mult)
                for j in range(8):
                    nc.vector.scalar_tensor_tensor(
                        out=wt_w[:, j, :], in0=plane(lo, hi, j), scalar=-8.0,
                        in1=srep[:], op0=ALU.add, op1=ALU.mult,
                    )

            # ---- phase B-X: quantize + transpose local x slice, AllGather ----
            xga = []
            for mt in range(MT):
                xloc = dram.tile([K, 128], FP8, tag=f"xloc{mt}")
                xga_mt = dram.tile([CORES * K, 128], FP8, tag=f"xga{mt}", addr_space="Shared")
                xga.append(xga_mt)
                xt_sb = xap.tile([128, KP, 128], FP8, tag="xt_sb")
                for h in range(NXT):
                    xa = xap.tile([128, K4], F32, tag="xa")
                    nc.sync.dma_start(
                        out=xa[:], in_=xs[mt * 128:(mt + 1) * 128, h * K4:(h + 1) * K4]
                    )
                    for wl in range(WPK4):
                        w = h * WPK4 + wl
                        for j in range(8):
                            pt = psumt.tile([128, 128], F32, tag="pt")
                            nc.tensor.transpose(
                                pt[:], xa[:, wl * 1024 + j: (wl + 1) * 1024: 8], ident[:]
                            )
                            nc.scalar.mul(out=xt_sb[:, w * 8 + j, :], in_=pt[:], mul=rx[:])
                nc.sync.dma_start(
                    out=xloc[:].rearrange("(c p) m -> p c m", p=128), in_=xt_sb[:]
                )
                nc.gpsimd.collective_compute(
                    "AllGather", ALU.bypass,
                    replica_groups=[list(range(CORES))],
                    ins=[xloc[:].opt()], outs=[xga_mt[:].opt()],
                )

            # ---- GEMM ----
            DR = mybir.MatmulPerfMode.DoubleRow
            for mt in range(MT):
                for c in range(CORES):
                    b = c * MT + mt  # global m-tile index
                    xt_g = xtp.tile([128, KP, 128], FP8, tag="xtg")
                    nc.sync.dma_start(
                        out=xt_g[:],
                        in_=xga[mt][c * K:(c + 1) * K, :].rearrange("(c p) m -> p c m", p=128),
                    )
                    xgc = []
                    for q in range(CAUG // 128):
                        t = xgcp.tile([128, 128], F32, tag="xgc")
                        nc.sync.dma_start(
                            out=t[:], in_=xgt[q * 128:(q + 1) * 128, b * 128:(b + 1) * 128]
                        )
                        xgc.append(t)
                    pss = []
                    for _nb in range(NB):
                        ps_nb = psummm.tile([128, NBW], F32, tag="ps")
                        pss.append(ps_nb)
                    for t_i in range(KP // 2):
                        w, j = (2 * t_i) // 8, (2 * t_i) % 8
                        for nb in range(NB):
                            nc.tensor.matmul(
                                pss[nb][:],
                                lhsT=xt_g[:, 2 * t_i:2 * t_i + 2, :],
                                rhs=wt_sb[w][:, j:j + 2, nb * NBW:(nb + 1) * NBW],
                                start=(t_i == 0), stop=False, perf_mode=DR,
                            )
                    nq = CAUG // 128
                    for q in range(nq):
                        for nb in range(NB):
                            nc.tensor.matmul(
                                pss[nb][:],
                                lhsT=xgc[q][:],
                                rhs=wct_sb[q][:, nb * NBW:(nb + 1) * NBW],
                                start=False, stop=(q == nq - 1),
                            )
                    y_sb = ysbp.tile([128, NL], F32, tag="ysb")
                    for nb in range(NB):
                        nc.scalar.mul(
                            out=y_sb[:, nb * NBW:(nb + 1) * NBW], in_=pss[nb][:], mul=s4[:]
                        )
                    nc.sync.dma_start(out=y[b * 128:(b + 1) * 128, :], in_=y_sb[:])

    nc.compile()
    return nc


def shard_inputs(x, q_weight, q_scale_col, weight_cache, ind, bias, M, K, N, CAUG):
    NL = N // CORES
    MSL = M // CORES
    FPn = ind.shape[0]
    x = np.asarray(x, np.float32)
    xg = x[:, np.asarray(ind)]
    xgt = np.zeros((CAUG, M), np.float32)
    xgt[:FPn] = xg.T
    xgt[FPn] = 1.0
    in_maps = []
    for c in range(CORES):
        n0 = c * NL
        wct = np.zeros((CAUG, NL), np.float32)
        wct[:FPn] = np.asarray(weight_cache, np.float32)[n0:n0 + NL].T
        wct[FPn] = np.asarray(bias, np.float32)[n0:n0 + NL]
        in_maps.append({
            "xs": np.ascontiguousarray(x[c * MSL:(c + 1) * MSL]),
            "qwt": np.ascontiguousarray(np.asarray(q_weight, np.int32)[n0:n0 + NL].T),
            "sct": np.ascontiguousarray(np.asarray(q_scale_col, np.float32)[n0:n0 + NL].T),
            "xgt": xgt,
            "wct": wct,
        })
    return in_maps


_NC_CACHE = {}


def get_nc(M=4096, K=8192, N=8192, CAUG=384):
    key = (M, K, N, CAUG)
    if key not in _NC_CACHE:
        _NC_CACHE[key] = build_kernel(M, K, N, CAUG)
    return _NC_CACHE[key]


def kernel(x, q_weight, q_scale_col, weight_cache, ind, bias):
    M, K = x.shape
    N = q_weight.shape[0]
    CAUG = 384
    nc = get_nc(M, K, N, CAUG)
    in_maps = shard_inputs(x, q_weight, q_scale_col, weight_cache, ind, bias, M, K, N, CAUG)
    res = run_bass_kernel_spmd(nc, in_maps, core_ids=list(range(CORES)))
    return np.concatenate([res.results[c]["y"] for c in range(CORES)], axis=1)


if __name__ == "__main__":
    nc = build_kernel()
    print("build+compile ok")


# revision 13
# speedup vs baseline: 1.0533x; 1.0533x over previous
"""Trainium2 Bass kernel for nn_MixLinear_GEMM (int4-dequant -> dynamic fp8 GEMM + outlier correction).

Self-contained: kernel(**inputs) takes full inputs, shards across 8 NeuronCores
(tensor-parallel along out_features N), runs one SPMD Bass kernel with
collectives (AllReduce for global maxes, chunked AllGather for fp8-quantized
x^T), and returns the full [M, N] float32 output.

Math notes:
 - reference quantizes to OCP float8_e4m3fn (max 448). TRN2's fp8e4 has max 240,
   so we quantize v/2 instead (max 224) and fold the 2x2 into the output scale.
   For this problem's data every nonzero |v| is far above the subnormal region,
   so the halved TRN rounding is bit-identical to e4m3fn rounding of v.
 - x is quantized to the fp8 grid BEFORE the PE-based transpose: the PE reads
   fp32 as FP22 (truncates mantissa), which would perturb roundings, but
   fp8-grid values pass through FP22 exactly.
 - y = (Xq@Wq^T)*(sx*sw) + bias + x[:,ind]@wc^T  is computed as
   y = psum_main * s4 + ycorr,   s4 = 4*sx*sw
   where psum_main = sum_k (Xq/2)(Wq/2)  (fp8 DoubleRow matmuls) and
   ycorr = xg_aug @ wct_aug^T (f32 matmuls, bias folded in as an extra
   all-ones column) is computed up front into DRAM while the maxes are being
   reduced, to keep the TensorEngine busy during the prologue.
"""
import sys

if "/opt/trn_rl_repo" not in sys.path:
    sys.path.insert(0, "/opt/trn_rl_repo")

import numpy as np

import concourse.bass as bass
import concourse.mybir as mybir
import concourse.tile as tile
from concourse import bacc, bass_isa
from concourse.bass_utils import run_bass_kernel_spmd
from concourse.masks import make_identity

F32 = mybir.dt.float32
I32 = mybir.dt.int32
U8 = mybir.dt.uint8
FP8 = mybir.dt.float8e4
ALU = mybir.AluOpType
AXL = mybir.AxisListType

CORES = 8
GROUP = 128
FP8_HALF_MAX = 224.0  # TRN fp8e4 max is 240; reference e4m3fn max is 448


def build_kernel(M=4096, K=8192, N=8192, CAUG=384):
    """Build the SPMD Bass graph (one graph, runs identically on all 8 cores)."""
    NL = N // CORES          # local out_features
    MSL = M // CORES         # local x row-slice
    KP = K // 128            # number of 128-wide k' chunks
    NWIN = KP // 8           # qwt row windows of 128 (each yields 8 planes)
    MT = MSL // 128          # local m-subtiles
    MB = M // 128            # global m-subtiles
    NB = max(1, NL // 512)   # psum banks per m-tile
    NBW = min(NL, 512)       # psum bank width
    K4 = min(K, 2048)        # x staging tile width
    WPK4 = K4 // 1024        # windows per x staging tile
    NXT = K // K4            # x staging tiles per m-row-tile
    NQ = CAUG // 128
    assert KP % 8 == 0 and MSL % 128 == 0 and NL % NBW == 0 and CAUG % 128 == 0

    nc = bacc.Bacc("TRN2", target_bir_lowering=False, debug=False, num_devices=CORES)

    xs = nc.declare_dram_parameter("xs", [MSL, K], F32, isOutput=False)
    qwt = nc.declare_dram_parameter("qwt", [K // 8, NL], I32, isOutput=False)
    sct = nc.declare_dram_parameter("sct", [K // GROUP, NL], F32, isOutput=False)
    xgt = nc.declare_dram_parameter("xgt", [CAUG, M], F32, isOutput=False)
    wct = nc.declare_dram_parameter("wct", [CAUG, NL], F32, isOutput=False)
    y = nc.declare_dram_parameter("y", [M, NL], F32, isOutput=True)

    with tile.TileContext(nc) as tc:
        with (
            tc.tile_pool(name="const", bufs=1) as constp,
            tc.tile_pool(name="wt", bufs=1) as wtp,
            tc.tile_pool(name="stream", bufs=2) as streamp,
            tc.tile_pool(name="xa", bufs=2) as xap,
            tc.tile_pool(name="xt", bufs=2) as xtp,
            tc.tile_pool(name="ysb", bufs=2) as ysbp,
            tc.tile_pool(name="ycb", bufs=2) as ycbp,
            tc.tile_pool(name="xgc", bufs=4) as xgcp,
            tc.tile_pool(name="psum_t", bufs=2, space="PSUM") as psumt,
            tc.tile_pool(name="psum_mm", bufs=4, space="PSUM") as psummm,
            tc.tile_pool(name="dram", bufs=1, space="DRAM") as dram,
        ):
            ident = constp.tile([128, 128], F32, tag="ident")
            make_identity(nc, ident[:])

            # persistent accumulators / scalars
            xmax_cols = constp.tile([128, MT * NXT], F32, tag="xmax")
            wmax_cols = constp.tile([128, NWIN], F32, tag="wmax")
            gmax_sb = constp.tile([128, 2], F32, tag="gmax")
            rx = constp.tile([128, 1], F32, tag="rx")
            rw = constp.tile([128, 1], F32, tag="rw")
            s4 = constp.tile([128, 1], F32, tag="s4")
            tmp1 = constp.tile([128, 1], F32, tag="tmp1")
            tmp2 = constp.tile([128, 1], F32, tag="tmp2")
            neg8 = constp.tile([128, 1], F32, tag="neg8")
            nc.vector.memset(neg8[:], -8.0)
            lmax2 = constp.tile([128, 2], F32, tag="lmax2")
            lred = constp.tile([128, 2], F32, tag="lred")

            # -------- correction GEMM prefill: ycorr = xg_aug @ wct_aug^T ----
            # Runs first so the TensorEngine has work while DVE/ACT reduce the
            # maxes. Unscaled f32; added to the scaled main psum in the
            # epilogue. Bias rides along as the all-ones column of xg_aug.
            wct_sb = []
            for q in range(NQ):
                t = constp.tile([128, NL], F32, tag=f"wct{q}")
                nc.sync.dma_start(out=t[:], in_=wct[q * 128:(q + 1) * 128, :])
                wct_sb.append(t)
            ycorr = dram.tile([M, NL], F32, tag="ycorr")
            for b in range(MB):
                xgc = []
                for q in range(NQ):
                    t = xgcp.tile([128, 128], F32, tag="xgc")
                    nc.sync.dma_start(
                        out=t[:], in_=xgt[q * 128:(q + 1) * 128, b * 128:(b + 1) * 128]
                    )
                    xgc.append(t)
                yc_sb = ycbp.tile([128, NL], F32, tag="ycs")
                for nb in range(NB):
                    psc = psumt.tile([128, NBW], F32, tag="big")
                    for q in range(NQ):
                        nc.tensor.matmul(
                            psc[:], lhsT=xgc[q][:],
                            rhs=wct_sb[q][:, nb * NBW:(nb + 1) * NBW],
                            start=(q == 0), stop=(q == NQ - 1),
                        )
                    nc.scalar.copy(out=yc_sb[:, nb * NBW:(nb + 1) * NBW], in_=psc[:])
                nc.sync.dma_start(out=ycorr[b * 128:(b + 1) * 128, :], in_=yc_sb[:])

            # -------- phase A: local max |W| and max |x| ----------------------
            def load_window_planes(w, engine):
                """DMA window w of qwt, unpack to lo/hi nibble planes (packed int32)."""
                qa = streamp.tile([128, NL], I32, tag="qa")
                nc.sync.dma_start(out=qa[:], in_=qwt[w * 128:(w + 1) * 128, :])
                hi = streamp.tile([128, NL], I32, tag="hi")
                engine.tensor_scalar(hi[:], qa[:], 4, None, ALU.logical_shift_right)
                engine.tensor_scalar(hi[:], hi[:], 0x0F0F0F0F, None, ALU.bitwise_and)
                engine.tensor_scalar(qa[:], qa[:], 0x0F0F0F0F, None, ALU.bitwise_and)
                return qa, hi

            def load_srep(w):
                """Scale rows for window w, replicated 16x across partitions."""
                srep = streamp.tile([128, NL], F32, tag="srep")
                for g in range(8):
                    nc.sync.dma_start(
                        out=srep[g * 16:(g + 1) * 16, :],
                        in_=sct[w * 8 + g:w * 8 + g + 1, :].broadcast_to([16, NL]),
                    )
                return srep

            def plane(lo, hi, j):
                src = lo if j % 2 == 0 else hi
                b = j // 2
                return src[:].bitcast(U8)[:, b::4]

            for w in range(NWIN):
                lo, hi = load_window_planes(w, nc.vector)
                srep = load_srep(w)
                dmax = streamp.tile([128, NL], F32, tag="dmax")
                for j in range(8):
                    # |nib - 8| on the Scalar engine
                    if j == 0:
                        nc.scalar.activation(
                            out=dmax[:], in_=plane(lo, hi, j),
                            func=mybir.ActivationFunctionType.Abs, bias=neg8[:], scale=1.0,
                        )
                    else:
                        dev = streamp.tile([128, NL], F32, tag="dev")
                        nc.scalar.activation(
                            out=dev[:], in_=plane(lo, hi, j),
                            func=mybir.ActivationFunctionType.Abs, bias=neg8[:], scale=1.0,
                        )
                        nc.vector.tensor_tensor(dmax[:], dmax[:], dev[:], ALU.max)
                nc.vector.tensor_tensor(dmax[:], dmax[:], srep[:], ALU.mult)
                nc.vector.tensor_reduce(
                    out=wmax_cols[:, w:w + 1], in_=dmax[:],
                    axis=AXL.X, op=ALU.max, apply_absolute_value=True,
                )

            for mt in range(MT):
                for h in range(NXT):
                    xa = xap.tile([128, K4], F32, tag="xa")
                    nc.sync.dma_start(
                        out=xa[:], in_=xs[mt * 128:(mt + 1) * 128, h * K4:(h + 1) * K4]
                    )
                    col = mt * NXT + h
                    nc.vector.tensor_reduce(
                        out=xmax_cols[:, col:col + 1], in_=xa[:],
                        axis=AXL.X, op=ALU.max, apply_absolute_value=True,
                    )

            # -------- AllReduce(max) of (gx, gw), derived scales --------------
            nc.vector.tensor_reduce(
                out=lmax2[:, 0:1], in_=xmax_cols[:], axis=AXL.X,
                op=ALU.max, apply_absolute_value=True,
            )
            nc.vector.tensor_reduce(
                out=lmax2[:, 1:2], in_=wmax_cols[:], axis=AXL.X,
                op=ALU.max, apply_absolute_value=True,
            )
            nc.gpsimd.partition_all_reduce(lred[:], lmax2[:], 128, bass_isa.ReduceOp.max)
            ar_in = dram.tile([1, 2], F32, tag="ar_in")
            ar_out = dram.tile([1, 2], F32, tag="ar_out")
            nc.sync.dma_start(out=ar_in[:], in_=lred[0:1, :])
            nc.gpsimd.collective_compute(
                "AllReduce", ALU.max,
                replica_groups=[list(range(CORES))],
                ins=[ar_in[:].opt()], outs=[ar_out[:].opt()],
            )
            g1 = constp.tile([1, 2], F32, tag="g1")
            nc.sync.dma_start(out=g1[:], in_=ar_out[:])
            nc.gpsimd.partition_broadcast(gmax_sb[:], g1[0:1, :], channels=128)

            # rx = 224/gx, rw = 224/gw, s4 = 4*sx*sw = gx*gw/50176
            # (DVE reciprocal is approximate; two Newton steps make it exact to
            #  f32 so the fp8 rounding boundaries match the reference's x/sx.)
            def refined_recip(out, g_ap):
                nc.vector.reciprocal(tmp1[:], g_ap)
                for _ in range(2):
                    nc.vector.tensor_tensor(tmp2[:], g_ap, tmp1[:], ALU.mult)
                    nc.vector.tensor_scalar(tmp2[:], tmp2[:], -1.0, 2.0, ALU.mult, ALU.add)
                    nc.vector.tensor_tensor(tmp1[:], tmp1[:], tmp2[:], ALU.mult)
                nc.vector.tensor_scalar(out, tmp1[:], FP8_HALF_MAX, None, ALU.mult)

            refined_recip(rx[:], gmax_sb[:, 0:1])
            refined_recip(rw[:], gmax_sb[:, 1:2])
            nc.vector.tensor_tensor(s4[:], gmax_sb[:, 0:1], gmax_sb[:, 1:2], ALU.mult)
            nc.vector.tensor_scalar(s4[:], s4[:], 1.0 / 50176.0, None, ALU.mult)

            # -------- phase B-X: quantize + transpose x slice, AllGather ------
            # fp8-quantize first (ACT), upcast back to f32 (exact), then PE
            # transpose: fp8-grid values survive the PE's FP22 read exactly.
            xga = []
            for mt in range(MT):
                xloc = dram.tile([K, 128], FP8, tag=f"xloc{mt}")
                xga_mt = dram.tile([CORES * K, 128], FP8, tag=f"xga{mt}", addr_space="Shared")
                xga.append(xga_mt)
                xt_sb = xap.tile([128, KP, 128], FP8, tag="xt_sb")
                for h in range(NXT):
                    xa = xap.tile([128, K4], F32, tag="xa")
                    nc.sync.dma_start(
                        out=xa[:], in_=xs[mt * 128:(mt + 1) * 128, h * K4:(h + 1) * K4]
                    )
                    xq8 = xap.tile([128, K4], FP8, tag="xq8")
                    nc.scalar.mul(out=xq8[:], in_=xa[:], mul=rx[:])
                    xb = xap.tile([128, K4], F32, tag="xa")
                    nc.scalar.copy(out=xb[:], in_=xq8[:])
                    for wl in range(WPK4):
                        w = h * WPK4 + wl
                        for j in range(8):
                            pt = psumt.tile([128, 128], F32, tag="pt")
                            nc.tensor.transpose(
                                pt[:], xb[:, wl * 1024 + j: (wl + 1) * 1024: 8], ident[:]
                            )
                            nc.scalar.copy(out=xt_sb[:, w * 8 + j, :], in_=pt[:])
                nc.sync.dma_start(
                    out=xloc[:].rearrange("(c p) m -> p c m", p=128), in_=xt_sb[:]
                )
                nc.gpsimd.collective_compute(
                    "AllGather", ALU.bypass,
                    replica_groups=[list(range(CORES))],
                    ins=[xloc[:].opt()], outs=[xga_mt[:].opt()],
                )

            # -------- phase B-W: quantize weights into resident Wt tiles ------
            wt_sb = []
            for w in range(NWIN):
                wt_w = wtp.tile([128, 8, NL], FP8, tag=f"wt{w}")
                wt_sb.append(wt_w)
                lo, hi = load_window_planes(w, nc.vector)
                srep = load_srep(w)
                nc.vector.tensor_scalar(srep[:], srep[:], rw[:], None, ALU.mult)
                for j in range(8):
                    nc.vector.scalar_tensor_tensor(
                        out=wt_w[:, j, :], in0=plane(lo, hi, j), scalar=-8.0,
                        in1=srep[:], op0=ALU.add, op1=ALU.mult,
                    )

            # -------- main GEMM: fp8 DoubleRow, epilogue adds ycorr -----------
            DR = mybir.MatmulPerfMode.DoubleRow
            for mt in range(MT):
                for c in range(CORES):
                    b = c * MT + mt  # global m-tile index
                    xt_g = xtp.tile([128, KP, 128], FP8, tag="xtg")
                    nc.sync.dma_start(
                        out=xt_g[:],
                        in_=xga[mt][c * K:(c + 1) * K, :].rearrange("(c p) m -> p c m", p=128),
                    )
                    ycb = ycbp.tile([128, NL], F32, tag="ycb")
                    nc.sync.dma_start(out=ycb[:], in_=ycorr[b * 128:(b + 1) * 128, :])
                    pss = []
                    for _nb in range(NB):
                        ps_nb = psummm.tile([128, NBW], F32, tag="ps")
                        pss.append(ps_nb)
                    for t_i in range(KP // 2):
                        w, j = (2 * t_i) // 8, (2 * t_i) % 8
                        for nb in range(NB):
                            nc.tensor.matmul(
                                pss[nb][:],
                                lhsT=xt_g[:, 2 * t_i:2 * t_i + 2, :],
                                rhs=wt_sb[w][:, j:j + 2, nb * NBW:(nb + 1) * NBW],
                                start=(t_i == 0), stop=(t_i == KP // 2 - 1),
                                perf_mode=DR,
                            )
                    y_sb = ysbp.tile([128, NL], F32, tag="ysb")
                    for nb in range(NB):
                        sl = slice(nb * NBW, (nb + 1) * NBW)
                        nc.vector.scalar_tensor_tensor(
                            out=y_sb[:, sl], in0=pss[nb][:], scalar=s4[:],
                            in1=ycb[:, sl], op0=ALU.mult, op1=ALU.add,
                        )
                    nc.sync.dma_start(out=y[b * 128:(b + 1) * 128, :], in_=y_sb[:])

    nc.compile()
    return nc


def shard_inputs(x, q_weight, q_scale_col, weight_cache, ind, bias, M, K, N, CAUG):
    NL = N // CORES
    MSL = M // CORES
    FPn = ind.shape[0]
    x = np.asarray(x, np.float32)
    xg = x[:, np.asarray(ind)]
    xgt = np.zeros((CAUG, M), np.float32)
    xgt[:FPn] = xg.T
    xgt[FPn] = 1.0
    in_maps = []
    for c in range(CORES):
        n0 = c * NL
        wct = np.zeros((CAUG, NL), np.float32)
        wct[:FPn] = np.asarray(weight_cache, np.float32)[n0:n0 + NL].T
        wct[FPn] = np.asarray(bias, np.float32)[n0:n0 + NL]
        in_maps.append({
            "xs": np.ascontiguousarray(x[c * MSL:(c + 1) * MSL]),
            "qwt": np.ascontiguousarray(np.asarray(q_weight, np.int32)[n0:n0 + NL].T),
            "sct": np.ascontiguousarray(np.asarray(q_scale_col, np.float32)[n0:n0 + NL].T),
            "xgt": xgt,
            "wct": wct,
        })
    return in_maps


_NC_CACHE = {}


def get_nc(M=4096, K=8192, N=8192, CAUG=384):
    key = (M, K, N, CAUG)
    if key not in _NC_CACHE:
        _NC_CACHE[key] = build_kernel(M, K, N, CAUG)
    return _NC_CACHE[key]


def kernel(x, q_weight, q_scale_col, weight_cache, ind, bias):
    M, K = x.shape
    N = q_weight.shape[0]
    CAUG = 384
    nc = get_nc(M, K, N, CAUG)
    in_maps = shard_inputs(x, q_weight, q_scale_col, weight_cache, ind, bias, M, K, N, CAUG)
    res = run_bass_kernel_spmd(nc, in_maps, core_ids=list(range(CORES)))
    return np.concatenate([res.results[c]["y"] for c in range(CORES)], axis=1)


if __name__ == "__main__":
    nc = build_kernel()
    print("build+compile ok")
